# revision 1
# baseline (speedup 1.0000x reference)
"""Trainium2 Bass kernel for DPL safe-policy head.

Computes, for x:[B,H] and three tiny heads Wg/Wp/Wa (4/4/5 logits):
    ghost  = softmax(x@Wg + bg); pacman = softmax(x@Wp + bp); base = softmax(x@Wa + ba)
    unsafe[b,a] = sum_cd pacman[b,c] * T[a,c,d] * ghost[b,d]   (T fixed 0/1 tensor)
    out = base*(1-unsafe) / sum(...)

Closed form used on device (softmax normalizations cancel except ghost/pacman's,
which fold into Sp*Sg):
    E = exp(logits), Sg = sum(EG), Sp = sum(EP), SS = Sp*Sg
    u0 = sum_c EPc*EGc ; u1 = EP0*EG1+EP2*EG3 ; u2 = EP1*EG0+EP3*EG2
    t_j = EA_j * (SS - u_j)  (u3 = u4 = 0);  out_j = t_j / sum_j t_j

Sharding: pure data parallel over batch across 8 cores (2048 rows each).

Per core pipeline (memory-bound target: stream x once from HBM):
  - x streams through the sync HWDGE queue as half-tile [128, 1024] DMAs
    (4 KiB lines, 8-deep buffering: ~320 GB/s measured)
  - PE transposes 128x128 fp32 chunks, 4 chunks packed per PSUM bank
  - PSUM->SBUF copy converts to fp16: hi = fp16(xT) on ACT; in split mode
    DVE also computes lo = fp16(xT - hi) for a 3-term exact matmul
  - fp16 matmuls (FWL weight loads) accumulate x@[Wg|Wp|Wa] + bias in PSUM
    (3-term mode: hiT@[Whi|Wlo] + loT@Whi recovers fp32-level precision);
    matmul emission lags transposes by one group so the in-order PE stream
    never stalls on the ACT/DVE copy chain
  - closed-form logic layer on DVE/ACT, two half-passes overlapping the loop
"""

import numpy as np

import concourse.bass as bass
import concourse.bacc as bacc
import concourse.mybir as mybir
import concourse.tile as tile
from concourse.bass_utils import run_bass_kernel_spmd

F32 = mybir.dt.float32
F16 = mybir.dt.float16
AX = mybir.AxisListType
ADD = mybir.AluOpType.add
SUB = mybir.AluOpType.subtract

MODE = "f16x3"  # one of: f16x3 (exact), f16x1 (fast)

N_CORES = 8
B_FULL, H = 16384, 2048
B = B_FULL // N_CORES  # rows per core
P = 128
NT = B // P            # batch tiles per core
NCH = H // P           # contraction chunks
GC = 4                 # chunks per psum transpose group (1 bank)
NG = NCH // GC
J = 13                 # 4 + 4 + 5 logits


def _build_program(mode):
    split = mode == "f16x3"
    mmdt = F16                    # storage dtype of matmul operands
    CW = 2 * J if split else J    # device-side W columns

    nc = bacc.Bacc("TRN2", target_bir_lowering=False, debug=False,
                   num_devices=N_CORES)
    x_d = nc.dram_tensor("x", [B, H], F32, kind="ExternalInput")
    w_d = nc.dram_tensor("w", [H, CW], mmdt, kind="ExternalInput")
    b_d = nc.dram_tensor("b", [1, CW], mmdt, kind="ExternalInput")
    e_d = nc.dram_tensor("ident", [P, P], F32, kind="ExternalInput")
    y_d = nc.dram_tensor("y", [B, 5], F32, kind="ExternalOutput")

    with tile.TileContext(nc) as tc:
        with (
            tc.tile_pool(name="const", bufs=1) as cpool,
            tc.tile_pool(name="xin", bufs=8) as xin_pool,
            tc.tile_pool(name="xt", bufs=4) as xt_pool,
            tc.tile_pool(name="tp", bufs=6, space="PSUM") as tp_pool,
            tc.tile_pool(name="acc", bufs=2, space="PSUM") as acc_pool,
            tc.tile_pool(name="work", bufs=1) as wpool,
            tc.tile_pool(name="tailp", bufs=2) as tpool,
        ):
            # ident + b are tiny and needed first: put them at the head of
            # the sync HWDGE queue (before x tile 0). The big strided w load
            # goes on the gpsimd SWDGE queue (its slow descriptor generation
            # overlaps the first transposes, which don't need w).
            id_sb = cpool.tile([P, P], F32)
            nc.sync.dma_start(id_sb[:], e_d.ap())
            b_sb = cpool.tile([1, CW], mmdt)
            nc.sync.dma_start(b_sb[:], b_d.ap())
            w_sb = cpool.tile([P, NCH, CW], mmdt)
            nc.gpsimd.dma_start(w_sb[:],
                                w_d.ap().rearrange("(c p) j -> p c j", p=P))
            ones_sb = cpool.tile([1, P], mmdt)
            nc.gpsimd.memset(ones_sb[:], 1.0)

            # logits staging, one tile per half so each half's tail only
            # depends on its own 8 batch tiles
            NTH = NT // 2
            all_st = [wpool.tile([P, NTH, J], F32, tag=f"st{h}",
                                 name=f"all_st{h}")
                      for h in range(2)]

            y3 = y_d.ap().rearrange("(t p) j -> p t j", p=P)

            def tail(h):
                st = all_st[h][:]
                e_all = tpool.tile([P, NTH, J], F32, tag="e_all")
                nc.scalar.activation(e_all[:], st,
                                     mybir.ActivationFunctionType.Exp)
                EG = e_all[:, :, 0:4]
                EP = e_all[:, :, 4:8]
                EA = e_all[:, :, 8:13]

                sg = tpool.tile([P, NTH], F32, tag="sg")
                nc.vector.tensor_reduce(sg[:], EG, axis=AX.X, op=ADD)
                sp = tpool.tile([P, NTH], F32, tag="sp")
                nc.vector.tensor_reduce(sp[:], EP, axis=AX.X, op=ADD)
                ss = tpool.tile([P, NTH], F32, tag="ss")
                nc.vector.tensor_mul(ss[:], sp[:], sg[:])

                tmp4 = tpool.tile([P, NTH, 4], F32, tag="tmp4")
                nc.vector.tensor_mul(tmp4[:], EP, EG)
                u3 = tpool.tile([P, NTH, 3], F32, tag="u3")
                nc.vector.tensor_reduce(u3[:, :, 0], tmp4[:], axis=AX.X,
                                        op=ADD)

                tmp2 = tpool.tile([P, NTH, 2], F32, tag="tmp2")
                nc.vector.tensor_mul(tmp2[:], e_all[:, :, 4:8:2],
                                     e_all[:, :, 1:4:2])
                nc.vector.tensor_reduce(u3[:, :, 1], tmp2[:], axis=AX.X,
                                        op=ADD)

                tmp2b = tpool.tile([P, NTH, 2], F32, tag="tmp2b")
                nc.vector.tensor_mul(tmp2b[:], e_all[:, :, 5:8:2],
                                     e_all[:, :, 0:3:2])
                nc.vector.tensor_reduce(u3[:, :, 2], tmp2b[:], axis=AX.X,
                                        op=ADD)

                V = tpool.tile([P, NTH, 5], F32, tag="V")
                nc.vector.tensor_sub(V[:, :, 0:3],
                                     ss[:].broadcast_to([P, NTH, 3]), u3[:])
                nc.vector.tensor_copy(V[:, :, 3:5],
                                      ss[:].broadcast_to([P, NTH, 2]))

                tj = tpool.tile([P, NTH, 5], F32, tag="tj")
                nc.vector.tensor_mul(tj[:], EA, V[:])
                s5 = tpool.tile([P, NTH], F32, tag="s5")
                nc.vector.tensor_reduce(s5[:], tj[:], axis=AX.X, op=ADD)
                r5 = tpool.tile([P, NTH], F32, tag="r5")
                nc.vector.reciprocal(r5[:], s5[:])

                out_sb = tpool.tile([P, NTH, 5], F32, tag="out_sb")
                nc.vector.tensor_mul(out_sb[:], tj[:],
                                     r5[:].broadcast_to([P, NTH, 5]))
                # out-DMA on the scalar HWDGE queue: keeps the strided
                # output descriptors out of the x-streaming sync queue
                nc.scalar.dma_start(y3[:, h * NTH:(h + 1) * NTH, :],
                                    out_sb[:])

            def fold(t, acc):
                if split:
                    # logits = (hi@Whi + lo@Whi) + hi@Wlo
                    # (both operands can't be PSUM: bounce one through SBUF)
                    tlo = xt_pool.tile([P, J], F32, tag="tlo")
                    nc.scalar.copy(tlo[:], acc[:, J:2 * J])
                    nc.vector.tensor_add(
                        all_st[t // NTH][:, t % NTH, :], acc[:, 0:J], tlo[:])
                else:
                    nc.scalar.copy(all_st[t // NTH][:, t % NTH, :], acc[:])
                if t % NTH == NTH - 1:
                    tail(t // NTH)

            def emit_matmuls(t, g, acc, hi8, lo8):
                for k in range(GC):
                    c = GC * g + k
                    last = c == NCH - 1
                    sl = slice(k * P, (k + 1) * P)
                    if split:
                        # lo@Whi adds into cols 0:13; hi@[Whi|Wlo] covers all
                        # 26 cols (emitted last so stop covers them)
                        nc.tensor.matmul(acc[:, 0:J], lo8[:, sl],
                                         w_sb[:, c, 0:J],
                                         start=False, stop=False,
                                         skip_group_check=True)
                    nc.tensor.matmul(acc[:], hi8[:, sl], w_sb[:, c, :],
                                     start=False, stop=last,
                                     skip_group_check=True)
                if g == NG - 1:
                    fold(t, acc)

            # Software-pipelined emission: each group's matmuls are emitted
            # one group AFTER its transposes, so the in-order PE stream does
            # group g+1's transposes while ACT/DVE produce group g's fp16
            # operands -- no PE stall waiting on the copy chain.
            pend = None

            for t in range(NT):
                # half-tile transfers (4 KiB lines): best balance of HBM
                # burst efficiency and pipeline granularity measured
                xq = []
                for q in range(2):
                    xqt = xin_pool.tile([P, H // 2], F32, tag=f"xh{q}",
                                        name=f"xh{t}_{q}")
                    nc.sync.dma_start(
                        xqt[:],
                        x_d.ap()[t * P:(t + 1) * P,
                                 q * (H // 2):(q + 1) * (H // 2)])
                    xq.append(xqt)

                def chunk(c, xq=xq):
                    h = NCH // 2
                    return xq[c // h][:, (c % h) * P:(c % h + 1) * P]

                acc = acc_pool.tile([P, CW], F32)
                # bias via rank-1 matmul: ones^T @ b broadcast; opens the group
                # (split mode: b is zero-padded to 26 cols so start covers all)
                nc.tensor.matmul(acc[:], ones_sb[:], b_sb[:],
                                 start=True, stop=False, skip_group_check=True)
                for g in range(NG):
                    tp = tp_pool.tile([P, GC * P], F32)
                    for k in range(GC):
                        c = GC * g + k
                        nc.tensor.transpose(
                            tp[:, k * P:(k + 1) * P],
                            chunk(c),
                            id_sb[:])
                    hi8 = xt_pool.tile([P, GC * P], mmdt, tag="hi")
                    if split:
                        nc.scalar.copy(hi8[:], tp[:])       # fp16 round on ACT
                        lo8 = xt_pool.tile([P, GC * P], F16, tag="lo")
                        nc.vector.tensor_tensor(lo8[:], tp[:], hi8[:], op=SUB)
                    else:
                        if g % 2 == 0:
                            nc.scalar.copy(hi8[:], tp[:])
                        else:
                            nc.vector.tensor_copy(hi8[:], tp[:])
                        lo8 = None
                    if pend is not None:
                        emit_matmuls(*pend)
                    pend = (t, g, acc, hi8, lo8)
            emit_matmuls(*pend)

    nc.compile()
    return nc


_NC_CACHE = {}


def _get_program(mode=MODE):
    if mode not in _NC_CACHE:
        _NC_CACHE[mode] = _build_program(mode)
    return _NC_CACHE[mode]


def _prep_in_maps(x, Wg, bg, Wp, bp, Wa, ba, mode=MODE):
    x = np.ascontiguousarray(np.asarray(x, dtype=np.float32))
    W = np.concatenate([np.asarray(Wg), np.asarray(Wp), np.asarray(Wa)],
                       axis=1).astype(np.float32)
    bvec = np.concatenate([np.asarray(bg), np.asarray(bp), np.asarray(ba)]
                          ).astype(np.float32).reshape(1, J)
    ident = np.eye(P, dtype=np.float32)
    if mode == "f16x3":
        Whi = W.astype(np.float16)
        Wlo = (W - Whi.astype(np.float32)).astype(np.float16)
        w_dev = np.concatenate([Whi, Wlo], axis=1)
        b_dev = np.concatenate([bvec, np.zeros_like(bvec)],
                               axis=1).astype(np.float16)
    else:  # f16x1
        w_dev = W.astype(np.float16)
        b_dev = bvec.astype(np.float16)
    in_maps = []
    for i in range(N_CORES):
        in_maps.append({
            "x": x[i * B:(i + 1) * B],
            "w": w_dev,
            "b": b_dev,
            "ident": ident,
        })
    return in_maps


def kernel(x, Wg, bg, Wp, bp, Wa, ba):
    in_maps = _prep_in_maps(x, Wg, bg, Wp, bp, Wa, ba)
    nc = _get_program()
    res = run_bass_kernel_spmd(nc, in_maps, core_ids=list(range(N_CORES)))
    return np.concatenate([res.results[i]["y"] for i in range(N_CORES)],
                          axis=0)



# revision 4
# speedup vs baseline: 1.1400x; 1.1400x over previous
"""Trainium2 Bass kernel for DPL safe-policy head.

Computes, for x:[B,H] and three tiny heads Wg/Wp/Wa (4/4/5 logits):
    ghost  = softmax(x@Wg + bg); pacman = softmax(x@Wp + bp); base = softmax(x@Wa + ba)
    unsafe[b,a] = sum_cd pacman[b,c] * T[a,c,d] * ghost[b,d]   (T fixed 0/1 tensor)
    out = base*(1-unsafe) / sum(...)

Closed form used on device (softmax normalizations cancel except ghost/pacman's,
which fold into Sp*Sg):
    E = exp(logits), Sg = sum(EG), Sp = sum(EP), SS = Sp*Sg
    u0 = sum_c EPc*EGc ; u1 = EP0*EG1+EP2*EG3 ; u2 = EP1*EG0+EP3*EG2
    t_j = EA_j * (SS - u_j)  (u3 = u4 = 0);  out_j = t_j / sum_j t_j

Sharding: pure data parallel over batch across 8 cores (2048 rows each).

Per core pipeline (memory-bound target: stream x once from HBM at ~320GB/s):
  - x streams through the sync HWDGE queue as half-tile [128, 1024] DMAs
    (4 KiB lines); identity/bias/weights ride other queues so the x stream
    owns the sync queue from t=0
  - ACT converts each half-tile to fp16 in SBUF (one pass over x)
  - PE transposes 128x128 fp16 chunks (1-pass, vs 2-pass LOW_HIGH for
    fp32), 4 chunks packed per PSUM bank
  - DVE copies PSUM->SBUF as fp16 transposed operands
  - one fp16 matmul per chunk accumulates x@[Wg|Wp|Wa] + bias in PSUM;
    matmul emission lags transposes by one group so the in-order PE stream
    never stalls on the DVE copy
  - closed-form logic layer on DVE/ACT, two half-passes overlapping the loop
  - output written as one contiguous [128, NT*5] block per half (320B
    partition lines); host reorders to [B, 5] (the old strided [.., t, j]
    store was 20B lines and cost a ~14us serial tail)

fp16 single-term matmul (f16x1): max rel err ~1.5e-3 vs the fp32 reference
(harness gate 2e-2).
"""

import numpy as np

import concourse.bass as bass
import concourse.bacc as bacc
import concourse.mybir as mybir
import concourse.tile as tile
from concourse.bass_utils import run_bass_kernel_spmd

F32 = mybir.dt.float32
F16 = mybir.dt.float16
AX = mybir.AxisListType
ADD = mybir.AluOpType.add
SUB = mybir.AluOpType.subtract

MODE = "f16pre"

N_CORES = 8
B_FULL, H = 16384, 2048
B = B_FULL // N_CORES  # rows per core
P = 128
NT = B // P            # batch tiles per core
NCH = H // P           # contraction chunks
GC = 4                 # chunks per psum transpose group (1 bank)
NG = NCH // GC
J = 13                 # 4 + 4 + 5 logits


def _build_program(mode):
    assert mode == "f16pre"
    nc = bacc.Bacc("TRN2", target_bir_lowering=False, debug=False,
                   num_devices=N_CORES)
    x_d = nc.dram_tensor("x", [B, H], F32, kind="ExternalInput")
    w_d = nc.dram_tensor("w", [H, J], F16, kind="ExternalInput")
    b_d = nc.dram_tensor("b", [1, J], F16, kind="ExternalInput")
    e_d = nc.dram_tensor("ident", [P, P], F16, kind="ExternalInput")
    y_d = nc.dram_tensor("y", [P, NT * 5], F32, kind="ExternalOutput")

    with tile.TileContext(nc) as tc:
        with (
            tc.tile_pool(name="const", bufs=1) as cpool,
            tc.tile_pool(name="xin", bufs=6) as xin_pool,
            tc.tile_pool(name="hi", bufs=6) as hi_pool,
            tc.tile_pool(name="xt", bufs=4) as xt_pool,
            tc.tile_pool(name="tp", bufs=6, space="PSUM") as tp_pool,
            tc.tile_pool(name="acc", bufs=2, space="PSUM") as acc_pool,
            tc.tile_pool(name="work", bufs=1) as wpool,
            tc.tile_pool(name="tailp", bufs=2) as tpool,
        ):
            # x owns the sync HWDGE queue from instruction 0. The tiny
            # identity rides the scalar queue (idle until the out-DMAs);
            # w/b go on the gpsimd SWDGE queue (slow descriptor generation
            # overlaps the first converts/transposes, which don't need w).
            id_sb = cpool.tile([P, P], F16)
            nc.scalar.dma_start(id_sb[:], e_d.ap())
            b_sb = cpool.tile([1, J], F16)
            nc.gpsimd.dma_start(b_sb[:], b_d.ap())
            w_sb = cpool.tile([P, NCH, J], F16)
            nc.gpsimd.dma_start(w_sb[:],
                                w_d.ap().rearrange("(c p) j -> p c j", p=P))
            ones_sb = cpool.tile([1, P], F16)
            nc.gpsimd.memset(ones_sb[:], 1.0)

            # logits staging, one tile per half so each half's tail only
            # depends on its own 8 batch tiles
            NTH = NT // 2
            all_st = [wpool.tile([P, NTH, J], F32, tag=f"st{h}",
                                 name=f"all_st{h}")
                      for h in range(2)]

            def tail(h):
                st = all_st[h][:]
                e_all = tpool.tile([P, NTH, J], F32, tag="e_all")
                nc.scalar.activation(e_all[:], st,
                                     mybir.ActivationFunctionType.Exp)
                EG = e_all[:, :, 0:4]
                EP = e_all[:, :, 4:8]
                EA = e_all[:, :, 8:13]

                sg = tpool.tile([P, NTH], F32, tag="sg")
                nc.vector.tensor_reduce(sg[:], EG, axis=AX.X, op=ADD)
                sp = tpool.tile([P, NTH], F32, tag="sp")
                nc.vector.tensor_reduce(sp[:], EP, axis=AX.X, op=ADD)
                ss = tpool.tile([P, NTH], F32, tag="ss")
                nc.vector.tensor_mul(ss[:], sp[:], sg[:])

                tmp4 = tpool.tile([P, NTH, 4], F32, tag="tmp4")
                nc.vector.tensor_mul(tmp4[:], EP, EG)
                u3 = tpool.tile([P, NTH, 3], F32, tag="u3")
                nc.vector.tensor_reduce(u3[:, :, 0], tmp4[:], axis=AX.X,
                                        op=ADD)

                tmp2 = tpool.tile([P, NTH, 2], F32, tag="tmp2")
                nc.vector.tensor_mul(tmp2[:], e_all[:, :, 4:8:2],
                                     e_all[:, :, 1:4:2])
                nc.vector.tensor_reduce(u3[:, :, 1], tmp2[:], axis=AX.X,
                                        op=ADD)

                tmp2b = tpool.tile([P, NTH, 2], F32, tag="tmp2b")
                nc.vector.tensor_mul(tmp2b[:], e_all[:, :, 5:8:2],
                                     e_all[:, :, 0:3:2])
                nc.vector.tensor_reduce(u3[:, :, 2], tmp2b[:], axis=AX.X,
                                        op=ADD)

                V = tpool.tile([P, NTH, 5], F32, tag="V")
                nc.vector.tensor_sub(V[:, :, 0:3],
                                     ss[:].broadcast_to([P, NTH, 3]), u3[:])
                nc.vector.tensor_copy(V[:, :, 3:5],
                                      ss[:].broadcast_to([P, NTH, 2]))

                tj = tpool.tile([P, NTH, 5], F32, tag="tj")
                nc.vector.tensor_mul(tj[:], EA, V[:])
                s5 = tpool.tile([P, NTH], F32, tag="s5")
                nc.vector.tensor_reduce(s5[:], tj[:], axis=AX.X, op=ADD)
                r5 = tpool.tile([P, NTH], F32, tag="r5")
                nc.vector.reciprocal(r5[:], s5[:])

                out_sb = tpool.tile([P, NTH, 5], F32, tag="out_sb")
                nc.vector.tensor_mul(out_sb[:], tj[:],
                                     r5[:].broadcast_to([P, NTH, 5]))
                # contiguous [128, 40] block (160B lines, 320B stride) on
                # the scalar HWDGE queue; host untangles the (t, p) order
                nc.scalar.dma_start(
                    y_d.ap()[:, h * NTH * 5:(h + 1) * NTH * 5],
                    out_sb[:].rearrange("p t j -> p (t j)"))

            def fold(t, acc):
                nc.scalar.copy(all_st[t // NTH][:, t % NTH, :], acc[:])
                if t % NTH == NTH - 1:
                    tail(t // NTH)

            def emit_matmuls(t, g, acc, xt):
                for k in range(GC):
                    c = GC * g + k
                    nc.tensor.matmul(acc[:], xt[:, k * P:(k + 1) * P],
                                     w_sb[:, c, :],
                                     start=False, stop=c == NCH - 1,
                                     skip_group_check=True)
                if g == NG - 1:
                    fold(t, acc)

            # Software-pipelined emission: each group's matmuls are emitted
            # one group AFTER its transposes, so the in-order PE stream does
            # group g+1's transposes while DVE copies group g's operands.
            pend = None

            for t in range(NT):
                # half-tile transfers (4 KiB lines): best balance of HBM
                # burst efficiency and pipeline granularity measured
                hq = []
                for q in range(2):
                    xqt = xin_pool.tile([P, H // 2], F32, tag=f"xh{q}",
                                        name=f"xh{t}_{q}")
                    nc.sync.dma_start(
                        xqt[:],
                        x_d.ap()[t * P:(t + 1) * P,
                                 q * (H // 2):(q + 1) * (H // 2)])
                    hqt = hi_pool.tile([P, H // 2], F16, tag=f"hh{q}",
                                       name=f"hh{t}_{q}")
                    nc.scalar.copy(hqt[:], xqt[:])  # fp16 round on ACT
                    hq.append(hqt)

                def chunk(c, hq=hq):
                    h = NCH // 2
                    return hq[c // h][:, (c % h) * P:(c % h + 1) * P]

                acc = acc_pool.tile([P, J], F32)
                # bias via rank-1 matmul: ones^T @ b broadcast; opens the
                # accumulation group
                nc.tensor.matmul(acc[:], ones_sb[:], b_sb[:],
                                 start=True, stop=False, skip_group_check=True)
                for g in range(NG):
                    tp = tp_pool.tile([P, GC * P], F16)
                    for k in range(GC):
                        c = GC * g + k
                        nc.tensor.transpose(
                            tp[:, k * P:(k + 1) * P],
                            chunk(c),
                            id_sb[:])
                    xt = xt_pool.tile([P, GC * P], F16, tag="xt")
                    nc.vector.tensor_copy(xt[:], tp[:])  # PSUM->SBUF fp16
                    if pend is not None:
                        emit_matmuls(*pend)
                    pend = (t, g, acc, xt)
            emit_matmuls(*pend)

    nc.compile()
    return nc


_NC_CACHE = {}


def _get_program(mode=MODE):
    if mode not in _NC_CACHE:
        _NC_CACHE[mode] = _build_program(mode)
    return _NC_CACHE[mode]


def _prep_in_maps(x, Wg, bg, Wp, bp, Wa, ba, mode=MODE):
    x = np.ascontiguousarray(np.asarray(x, dtype=np.float32))
    W = np.concatenate([np.asarray(Wg), np.asarray(Wp), np.asarray(Wa)],
                       axis=1).astype(np.float32)
    bvec = np.concatenate([np.asarray(bg), np.asarray(bp), np.asarray(ba)]
                          ).astype(np.float32).reshape(1, J)
    ident = np.eye(P, dtype=np.float16)
    w_dev = W.astype(np.float16)
    b_dev = bvec.astype(np.float16)
    in_maps = []
    for i in range(N_CORES):
        in_maps.append({
            "x": x[i * B:(i + 1) * B],
            "w": w_dev,
            "b": b_dev,
            "ident": ident,
        })
    return in_maps


def kernel(x, Wg, bg, Wp, bp, Wa, ba):
    in_maps = _prep_in_maps(x, Wg, bg, Wp, bp, Wa, ba)
    nc = _get_program()
    res = run_bass_kernel_spmd(nc, in_maps, core_ids=list(range(N_CORES)))
    outs = []
    for i in range(N_CORES):
        y = res.results[i]["y"]  # [P, NT*5], row b = t*P + p at [p, t*5+j]
        outs.append(y.reshape(P, NT, 5).transpose(1, 0, 2).reshape(B, 5))
    return np.concatenate(outs, axis=0)
